# revision 82
# baseline (speedup 1.0000x reference)
"""Trainium2 Bass kernel for nn_Aggregator (NeRF-style point aggregation MLP).

Fully data-parallel over 8 NeuronCores: each core processes N/8 = 4096
samples (x16 neighbors = 65536 rows); weights replicated, no collectives.

Key design points:
- Activations stay feature-major ([features, rows]) so all MLP matmuls
  chain on the TensorEngine without transposes; host permutes weight rows
  to match the on-chip feature layout (see _w1_perm/_wc1_perm).
- All big matmuls run in float32r (1 PE cycle/row at N=512, vs 4 for fp32;
  storage is full fp32 bits, the PE rounds on read -> rel err ~3e-5).
- Positional encodings: phases = (R/2pi) @ [pcf; xyz; sp; xyz^2; sp^2] as
  ONE block-diagonal f32r matmul per chunk, fp32 magic-constant
  round-to-nearest range reduction on the DVE, then exactly two Sin table
  lookups per iteration (X-chunk layout groups all sin rows in XA and all
  cos rows in XB; the cos sign/phase shift is folded into the host-side
  weight rows and the activation bias).
- Inverse-distance weights are precomputed in a [128, 512]-packed stage
  (full-lane sqrt/reciprocal), normalized via segmented tensor_reduce, and
  broadcast per row-tile with a partition-stride-0 DMA from DRAM scratch.
- softplus/sigmoid are composed from exp/ln/relu so the whole kernel needs
  only three activation-table loads (sqrt -> sin -> natural_log_exp).
- The main loop is software-pipelined at quarter-iteration granularity:
  each emission round issues head(t+2) | tail(t-1) | L1(t+1) | L3(t) |
  L2(t+1) | L4(t), with per-half prelu issue inside each layer, so the
  PE/ACT/DVE/GPSIMD FIFOs never block on the same iteration's chain
  (timeline-sim: 3911us naive fp32 -> 876us final); emb_ray
  staging is front-loaded into the pipeline-fill rounds, and the Sin
  lookups are issued per-512-row chunk to shorten the staging chain.
- Per-neighbor K-reduction: weighted multiply on GPSIMD, segmented
  tensor_reduce on the DVE; alpha head padded to M=2 to stay f32r-legal.
"""
import math

import numpy as np

import concourse.bass as bass
import concourse.mybir as mybir
import concourse.tile as tile
from concourse import bacc
from concourse.bass_utils import run_bass_kernel_spmd

FP = mybir.dt.float32
FR = mybir.dt.float32r
A = mybir.ActivationFunctionType
OP = mybir.AluOpType

N_CORES = 8
N_FULL, K, F = 32768, 16, 32
NK = N_FULL * K
N_PER = N_FULL // N_CORES          # 4096 samples per core
R_PER = N_PER * K                  # 65536 rows per core
FD = 1024                          # rows per main-loop iteration
N_ITERS = R_PER // FD              # 64
NCH = N_PER // 512                 # 8 sample chunks (color / emb_ray)

TWO_PI = 2.0 * math.pi
INV_2PI = 1.0 / TWO_PI
MAGIC = 12582912.0                 # 1.5 * 2**23: fp32 round-to-nearest trick


# ---------------------------------------------------------------- host prep
def _w1_perm():
    # XA = [sinf(96) | sind(30)]; XB = [cosf(96) | cosd(30)]; XC = pcf(32)
    perm = np.empty(284, dtype=np.int64)
    for j in range(3):
        for i in range(32):
            perm[32 * j + i] = 32 + 2 * (3 * i + j)                # sinf
            perm[126 + 32 * j + i] = 32 + 2 * (3 * i + j) + 1      # cosf
    for j in range(5):
        for i in range(3):
            perm[96 + 3 * j + i] = 224 + 2 * (5 * i + j)               # sin d
            perm[96 + 15 + 3 * j + i] = 224 + 2 * (5 * (3 + i) + j)    # sin dp
            perm[222 + 3 * j + i] = 224 + 2 * (5 * i + j) + 1          # cos d
            perm[222 + 15 + 3 * j + i] = 224 + 2 * (5 * (3 + i) + j) + 1
    for i in range(32):
        perm[252 + i] = i
    return perm


def _wc1_perm():
    perm = np.empty(280, dtype=np.int64)
    perm[:256] = np.arange(256)
    for j in range(4):
        for i in range(3):
            perm[256 + 3 * j + i] = 256 + 2 * (4 * i + j)
            perm[256 + 12 + 3 * j + i] = 256 + 2 * (4 * i + j) + 1
    return perm


def _consts(inp):
    """Host-side constant tensors, packed into 3 arrays (3 DMAs)."""
    f32 = np.float32
    w1p = np.asarray(inp["w1"], f32)[_w1_perm()].copy()
    w1p[126:252] *= -1.0   # XB holds -cos: fold sign into cos weight rows
    w2 = np.asarray(inp["w2"], f32)
    # block3 extra features: Y2 = [dir(3), rgb(3), rays(3), prod(3)]
    w3 = np.asarray(inp["w3"], f32)
    w3p = np.zeros((268, 256), f32)
    w3p[:256] = w3[:256]
    w3p[256:259] = w3[259:262]      # dir
    w3p[259:262] = w3[256:259]      # rgb
    w3p[262:265] = -w3[259:262]     # rays
    w3p[265:268] = np.broadcast_to(w3[262:263], (3, 256))  # prod -> dot
    w4 = np.asarray(inp["w4"], f32)
    wa = np.asarray(inp["wa"], f32)           # [256, 1] -> pad M to 2
    wap = np.concatenate([wa, np.zeros((256, 1), f32)], axis=1)
    wc1p = np.asarray(inp["wc1"], f32)[_wc1_perm()]

    wpk = np.zeros((128, 3208), f32)
    blocks = [
        (w1p[0:126], 0), (w1p[126:252], 256), (w1p[252:284], 512),
        (w2[0:128], 768), (w2[128:256], 1024),
        (w3p[0:128], 1280), (w3p[128:256], 1536), (w3p[256:268], 1792),
        (w4[0:128], 2048), (w4[128:256], 2304),
        (wc1p[0:128], 2560), (wc1p[128:256], 2688), (wc1p[256:280], 2816),
        (np.asarray(inp["wc2"], f32), 2944), (np.asarray(inp["wc3"], f32), 3072),
        (np.asarray(inp["wc4"], f32), 3200),
        (wap[0:128], 3203), (wap[128:256], 3205),
    ]
    for arr, c0 in blocks:
        wpk[0:arr.shape[0], c0:c0 + arr.shape[1]] = arr

    bpk = np.zeros((128, 16), f32)
    b1, b2, b3, b4 = (np.asarray(inp[k], f32) for k in ("b1", "b2", "b3", "b4"))
    for col, v in enumerate((b1[0:128], b1[128:256], b2[0:128], b2[128:256],
                             b3[0:128], b3[128:256], b4[0:128], b4[128:256],
                             np.asarray(inp["bc1"], f32),
                             np.asarray(inp["bc2"], f32),
                             np.asarray(inp["bc3"], f32))):
        bpk[0:len(v), col] = v
    ba = float(np.asarray(inp["ba"], f32).reshape(-1)[0])
    bpk[:, 11] = ba - 1.0                               # bam1
    bpk[0:3, 12] = -np.asarray(inp["bc4"], f32)         # negbc4

    # staging matrices (phases divided by 2*pi for range reduction)
    rpk = np.zeros((44, 152), f32)
    for j in range(3):
        for i in range(32):
            rpk[i, 32 * j + i] = (2.0 ** j) * INV_2PI
    for j in range(5):
        for i in range(3):
            s = (2.0 ** j) * INV_2PI
            rpk[32 + i, 96 + 3 * j + i] = s        # xyz
            rpk[35 + i, 96 + 3 * j + i] = -s       # -sp
            rpk[38 + i, 96 + 15 + 3 * j + i] = s   # xyz^2
            rpk[41 + i, 96 + 15 + 3 * j + i] = -s  # -sp^2
    for j in range(4):
        for i in range(3):
            s = (2.0 ** j) * INV_2PI
            rpk[i, 126 + 3 * j + i] = s
            rpk[i, 126 + 12 + 3 * j + i] = s
            rpk[3, 126 + 12 + 3 * j + i] = 0.25    # cos phase (pre-rounding)
    return {"WPK": wpk, "BPK": bpk, "RPK": rpk}


def _shards(inp):
    """Per-core data arrays."""
    f32 = np.float32
    pc_xyz = np.asarray(inp["pc_xyz"], f32).reshape(NK, 3)
    pc_rgb = np.asarray(inp["pc_rgb"], f32).reshape(NK, 3)
    pc_dir = np.asarray(inp["pc_dir"], f32).reshape(NK, 3)
    pc_feat = np.asarray(inp["pc_feat"], f32).reshape(NK, F)
    sp = np.asarray(inp["output_sp_xyz"], f32)
    mask = np.asarray(inp["valid_sp_mask_out"]).astype(f32).reshape(NK)
    rays = np.asarray(inp["rays_d"], f32)
    sp_rep = np.repeat(sp, K, axis=0)
    rays_rep = np.repeat(rays, K, axis=0)
    xs_full = np.concatenate([pc_xyz.T, sp_rep.T], axis=0)     # [6, NK]
    ya_full = np.concatenate([pc_dir.T, pc_rgb.T], axis=0)     # [6, NK]

    shards = []
    for cidx in range(N_CORES):
        rs = slice(cidx * R_PER, (cidx + 1) * R_PER)
        ns = slice(cidx * N_PER, (cidx + 1) * N_PER)
        rays4 = np.concatenate([rays[ns].T, np.ones((1, N_PER), f32)], axis=0)
        shards.append({
            "pcfT": np.ascontiguousarray(pc_feat[rs].T),
            "XS": np.ascontiguousarray(xs_full[:, rs]),
            "YA": np.ascontiguousarray(ya_full[:, rs]),
            "raysRT": np.ascontiguousarray(rays_rep[rs].T),
            "maskF": np.ascontiguousarray(mask[rs]),
            "raysS4": np.ascontiguousarray(rays4),
        })
    return shards


# ---------------------------------------------------------------- device build
_CACHE = {}


def _build():
    nc = bacc.Bacc("TRN2", target_bir_lowering=False, debug=False)

    h = {}
    data_specs = {
        "pcfT": ([F, R_PER], FR), "XS": ([6, R_PER], FR),
        "YA": ([6, R_PER], FR), "raysRT": ([3, R_PER], FR),
        "maskF": ([R_PER], FP), "raysS4": ([4, N_PER], FR),
    }
    const_specs = {
        "WPK": ([128, 3208], FR), "BPK": ([128, 16], FP),
        "RPK": ([44, 152], FR),
    }
    for name, (shp, dt) in {**data_specs, **const_specs}.items():
        h[name] = nc.dram_tensor(name, shp, dt, kind="ExternalInput")
    out_h = nc.dram_tensor("out", [4, N_PER], FP, kind="ExternalOutput")
    h["er_scr"] = nc.dram_tensor("er_scr", [24, N_PER], FP)
    h["wscr"] = nc.dram_tensor("wscr", [R_PER], FR)
    h["z_scr"] = nc.dram_tensor("z_scr", [R_PER], FP)

    with tile.TileContext(nc) as tc:
        _program(nc, tc, h, out_h)
    nc.compile()
    return nc


def _program(nc, tc, h, out_h):
    ctxs = []

    def pool(name, bufs, space="SBUF"):
        p = tc.tile_pool(name=name, bufs=bufs, space=space)
        ctxs.append(p)
        return p.__enter__()

    consts = pool("consts", 1)
    persist = pool("persist", 1)

    wpk = consts.tile([128, 3208], FR)
    nc.sync.dma_start(wpk[:], h["WPK"][:])
    bpk = consts.tile([128, 16], FP)
    nc.sync.dma_start(bpk[:], h["BPK"][:])
    rpk = consts.tile([44, 152], FR)
    nc.sync.dma_start(rpk[:], h["RPK"][:])
    cs = {}
    for name, rows, c0, w in (
            ("w1c0", 126, 0, 256), ("w1c1", 126, 256, 256),
            ("w1c2", 32, 512, 256), ("w2c0", 128, 768, 256),
            ("w2c1", 128, 1024, 256), ("w3c0", 128, 1280, 256),
            ("w3c1", 128, 1536, 256), ("w3c2", 12, 1792, 256),
            ("w4c0", 128, 2048, 256), ("w4c1", 128, 2304, 256),
            ("wc1c0", 128, 2560, 128), ("wc1c1", 128, 2688, 128),
            ("wc1c2", 24, 2816, 128), ("wc2", 128, 2944, 128),
            ("wc3", 128, 3072, 128), ("wc4", 128, 3200, 3),
            ("wac0", 128, 3203, 2), ("wac1", 128, 3205, 2)):
        cs[name] = wpk[0:rows, c0:c0 + w]
    for col, name in enumerate(("b1c0", "b1c1", "b2c0", "b2c1", "b3c0",
                                "b3c1", "b4c0", "b4c1", "bc1", "bc2",
                                "bc3", "bam1")):
        cs[name] = bpk[0:128, col:col + 1]
    cs["negbc4"] = bpk[0:3, 12:13]
    cs["Rcomb"] = rpk[0:44, 0:126]
    cs["Rr2"] = rpk[0:4, 126:150]
    ones_r = consts.tile([1, 128], FR)
    nc.gpsimd.memset(ones_r[:].bitcast(FP), 1.0)
    negpi2 = consts.tile([128, 1], FP)
    nc.gpsimd.memset(negpi2[:], -math.pi / 2)

    weights_all = persist.tile([128, 512], FR)   # row t2 = 512-row tile t2
    FA0 = persist.tile([128, N_PER], FR)
    FA1 = persist.tile([128, N_PER], FR)

    def row_pf(i):
        # row i of XS [6, R_PER], viewed as [128, 512] fp32
        return h["XS"][i:i + 1, :].rearrange(
            "o (p f) -> (o p) f", f=512).bitcast(FP)

    # ================= stage 1: inverse-distance weights (sqrt table) ======
    with tc.tile_pool(name="s1", bufs=1) as s1:
        g = {}
        for nm, i in (("gx0", 0), ("gx1", 1), ("gx2", 2),
                      ("gs0", 3), ("gs1", 4), ("gs2", 5)):
            t = s1.tile([128, 512], FP, name=nm)
            nc.sync.dma_start(t[:], row_pf(i))
            g[nm] = t
        gm = s1.tile([128, 512], FP)
        nc.sync.dma_start(gm[:],
                          h["maskF"][:].rearrange("(p f) -> p f", f=512))

        d2 = s1.tile([128, 512], FP)
        sqs = []
        for i in range(3):
            df = s1.tile([128, 512], FP, name=f"df{i}")
            nc.vector.tensor_sub(df[:], g[f"gx{i}"][:], g[f"gs{i}"][:])
            sq = s1.tile([128, 512], FP, name=f"sq{i}")
            nc.scalar.activation(sq[:], df[:], A.Square)
            sqs.append(sq)
        nc.vector.tensor_add(d2[:], sqs[0][:], sqs[1][:])
        nc.vector.tensor_add(d2[:], d2[:], sqs[2][:])
        dn = s1.tile([128, 512], FP)
        nc.scalar.activation(dn[:], d2[:], A.Sqrt)
        nc.vector.tensor_scalar_add(dn[:], dn[:], 1e-6)
        rec = s1.tile([128, 512], FP)
        nc.vector.reciprocal(rec[:], dn[:])
        inv = s1.tile([128, 512], FP)
        nc.vector.tensor_mul(inv[:], rec[:], gm[:])
        wsum = s1.tile([128, 32], FP)
        nc.vector.tensor_reduce(wsum[:],
                                inv[:].rearrange("p (a b) -> p a b", b=16),
                                axis=mybir.AxisListType.X, op=OP.add)
        nc.vector.tensor_scalar_add(wsum[:], wsum[:], 1e-6)
        wr = s1.tile([128, 32], FP)
        nc.vector.reciprocal(wr[:], wsum[:])
        wrep = s1.tile([128, 512], FP)
        wrep_ap = wrep[:]
        for k in range(16):
            dst = bass.AP(tensor=wrep_ap.tensor, offset=wrep_ap.offset + k,
                          ap=[wrep_ap.ap[0], [16, 32]])
            nc.vector.tensor_copy(dst, wr[:])
        nc.vector.tensor_mul(weights_all[:].bitcast(FP), inv[:], wrep[:])
        nc.sync.dma_start(
            h["wscr"][:].rearrange("(p f) -> p f", f=512), weights_all[:])

    # ================= stage 2: main loop (sin table) ======================
    # ---- main-loop pools (opened early so the first heads can be emitted
    # before stage 1, hiding stage-1 latency under the loop start)
    main_ctx = [
        tc.tile_pool(name="main", bufs=2),
        tc.tile_pool(name="mtmp", bufs=1),
        tc.tile_pool(name="psS", bufs=2, space="PSUM"),
        tc.tile_pool(name="psMLP", bufs=3, space="PSUM"),
    ]
    main, mtmp, psS_pool, psMLP = (p.__enter__() for p in main_ctx)
    psZ_pool = psS_pool

    def stage_head(it):
        r0 = it * FD
        rslc = bass.ds(r0, FD)
        T = {}
        T["XA"] = main.tile([126, FD], FR, tag="XA", name="XA")
        T["XB"] = main.tile([126, FD], FR, tag="XB", name="XB")
        T["XC"] = main.tile([32, FD], FR, tag="XC", name="XC")
        SRC = main.tile([44, FD], FR, tag="SRC", name="SRC")
        U2 = main.tile([6, FD], FR, tag="U2", bufs=2, name="U2")
        T["Y2"] = main.tile([12, FD], FR, tag="Y2", bufs=3, name="Y2")
        R3 = main.tile([3, FD], FR, tag="R3", bufs=2, name="R3")
        P3 = main.tile([3, FD], FR, tag="P3", bufs=2, name="P3")
        Y2 = T["Y2"]

        nc.sync.dma_start(SRC[0:32, :], h["pcfT"][:, rslc])
        nc.sync.dma_start(T["XC"][:], h["pcfT"][:, rslc])
        nc.sync.dma_start(SRC[32:38, :], h["XS"][:, rslc])
        nc.sync.dma_start(Y2[0:6, :], h["YA"][:, rslc])
        nc.sync.dma_start(R3[:], h["raysRT"][:, rslc])
        nc.sync.dma_start(Y2[6:9, :], R3[:])
        nc.gpsimd.tensor_tensor(U2[:].bitcast(FP), SRC[32:38, :].bitcast(FP),
                                SRC[32:38, :].bitcast(FP), op=OP.mult)
        nc.sync.dma_start(SRC[38:44, :], U2[:])
        nc.gpsimd.tensor_tensor(P3[:].bitcast(FP), Y2[0:3, :].bitcast(FP),
                                R3[:].bitcast(FP), op=OP.mult)
        nc.sync.dma_start(Y2[9:12, :], P3[:])

        dS = mtmp.tile([126, FD], FP, tag="dS", bufs=2, name="dS")
        dC = mtmp.tile([126, FD], FP, tag="dC", bufs=2, name="dC")
        for n in range(2):
            nslc = bass.ts(n, 512)
            psS = psS_pool.tile([126, 512], FP, tag="psS", name="psS")
            nc.tensor.matmul(psS[:], cs["Rcomb"][:], SRC[:, nslc],
                             start=True, stop=True)
            nfS = mtmp.tile([126, 512], FP, tag="nfS", bufs=2, name="nfS")
            nc.vector.tensor_scalar(nfS[:], psS[:], MAGIC, MAGIC,
                                    op0=OP.add, op1=OP.subtract)
            nc.vector.tensor_sub(dS[:, nslc], psS[:], nfS[:])
            # dC' = (dS >= 0.25) - dS;  XB = sin(2*pi*dC' - pi/2) = -cos(2*pi*dS)
            nc.vector.scalar_tensor_tensor(dC[:, nslc], dS[:, nslc], 0.25,
                                           dS[:, nslc], op0=OP.is_ge,
                                           op1=OP.subtract)
        for n in range(2):
            nslc = bass.ts(n, 512)
            nc.scalar.activation(T["XA"][:, nslc], dS[:, nslc], A.Sin,
                                 scale=TWO_PI)
            nc.scalar.activation(T["XB"][:, nslc], dC[:, nslc], A.Sin,
                                 scale=TWO_PI, bias=negpi2[:126, 0:1])
        if it < NCH:
            c = it
            cslc = bass.ts(c, 512)
            r4 = mtmp.tile([4, 512], FR, tag="r4", bufs=1, name="r4")
            nc.sync.dma_start(r4[:], h["raysS4"][:, cslc])
            psER = psS_pool.tile([24, 512], FP, tag="psS", name="psER")
            nc.tensor.matmul(psER[:], cs["Rr2"][:], r4[:],
                             start=True, stop=True)
            nfR = mtmp.tile([24, 512], FP, tag="nfR2", bufs=1, name="nfR2")
            nc.vector.tensor_scalar(nfR[:], psER[:], MAGIC, MAGIC,
                                    op0=OP.add, op1=OP.subtract)
            dR = mtmp.tile([24, 512], FP, tag="dR", bufs=1, name="dR")
            nc.vector.tensor_sub(dR[:], psER[:], nfR[:])
            erc = mtmp.tile([24, 512], FP, tag="erc", bufs=1, name="erc")
            nc.scalar.activation(erc[:], dR[:], A.Sin, scale=TWO_PI)
            nc.sync.dma_start(h["er_scr"][:, cslc], erc[:])
        return T

    heads = stage_head(0)
    nexts = stage_head(1)

    def body_tail(it, F4a, F4b):
        # alpha preactivation + weighted aggregation for iteration `it`;
        # emitted one iteration late so the engine FIFOs never wait on the
        # current iteration's last prelu.
        for n in range(2):
            nslc = bass.ts(n, 512)
            t2 = 2 * it + n
            psZ = psZ_pool.tile([2, 512], FP, tag="psS", name="psZ")
            nc.tensor.matmul(psZ[:], cs["wac0"][:], F4a[:, nslc],
                             start=True, stop=False)
            nc.tensor.matmul(psZ[:], cs["wac1"][:], F4b[:, nslc],
                             start=False, stop=True)
            ztmp = mtmp.tile([1, 512], FP, tag="ztmp", bufs=2, name="ztmp")
            nc.vector.tensor_copy(ztmp[:], psZ[0:1, :])
            nc.sync.dma_start(h["z_scr"][t2 * 512:(t2 + 1) * 512], ztmp[:])

            wb = mtmp.tile([128, 512], FP, tag="wb", bufs=2, name="wb")
            nc.sync.dma_start(wb[:], bass.AP(
                tensor=h["wscr"], offset=t2 * 512, ap=[[0, 128], [1, 512]]
            ).bitcast(FP))
            sout = bass.ds(it * 64 + n * 32, 32)
            for fa_dst, f4 in ((FA0, F4a), (FA1, F4b)):
                tmp = mtmp.tile([128, 512], FP, tag="aggtmp", bufs=2,
                                name="aggtmp")
                nc.gpsimd.tensor_tensor(tmp[:], f4[:, nslc].bitcast(FP),
                                        wb[:], op=OP.mult)
                with nc.allow_low_precision(reason="f32r is full fp32 bits"):
                    nc.vector.tensor_reduce(
                        fa_dst[:, sout],
                        tmp[:].rearrange("p (a b) -> p a b", b=16),
                        axis=mybir.AxisListType.X, op=OP.add)

    def mlp_layer(srcs, wts, bias, outs, tagp, split_act=False):
        ps = [psMLP.tile([128, FD], FP, tag="mlp", name=tagp + "a"),
              psMLP.tile([128, FD], FP, tag="mlp", name=tagp + "b")]
        for half in range(2):
            for n in range(2):
                nslc = bass.ts(n, 512)
                for ci, (src, wt) in enumerate(zip(srcs, wts)):
                    nc.tensor.matmul(
                        ps[half][:, nslc],
                        wt[:, bass.ts(half, 128)],
                        src[:, nslc],
                        start=(ci == 0), stop=(ci == len(srcs) - 1))
                if split_act:
                    nc.scalar.activation(outs[half][:, nslc],
                                         ps[half][:, nslc], A.Prelu,
                                         bias=bias[half][:, 0:1], alpha=0.01)
            if not split_act:
                nc.scalar.activation(outs[half][:], ps[half][:],
                                     A.Prelu, bias=bias[half][:, 0:1],
                                     alpha=0.01)

    def emit_L1(T):
        F1a = main.tile([128, FD], FR, tag="F1a", bufs=1, name="F1a")
        F1b = main.tile([128, FD], FR, tag="F1b", bufs=1, name="F1b")
        mlp_layer([T["XA"][:], T["XB"][:], T["XC"][:]],
                  [cs["w1c0"], cs["w1c1"], cs["w1c2"]],
                  [cs["b1c0"], cs["b1c1"]], [F1a, F1b], "L1",
                  split_act=True)
        return F1a, F1b

    def emit_L2(F1a, F1b):
        Y0 = main.tile([128, FD], FR, tag="Y0", bufs=2, name="Y0")
        Y1 = main.tile([128, FD], FR, tag="Y1", bufs=2, name="Y1")
        mlp_layer([F1a[:], F1b[:]], [cs["w2c0"], cs["w2c1"]],
                  [cs["b2c0"], cs["b2c1"]], [Y0, Y1], "L2")
        return Y0, Y1

    def emit_L3(T, Y0, Y1):
        F3a = main.tile([128, FD], FR, tag="F3a", bufs=1, name="F3a")
        F3b = main.tile([128, FD], FR, tag="F3b", bufs=1, name="F3b")
        mlp_layer([Y0[:], Y1[:], T["Y2"][:]],
                  [cs["w3c0"], cs["w3c1"], cs["w3c2"]],
                  [cs["b3c0"], cs["b3c1"]], [F3a, F3b], "L3")
        return F3a, F3b

    def emit_L4(F3a, F3b):
        F4a = main.tile([128, FD], FR, tag="F4a", bufs=2, name="F4a")
        F4b = main.tile([128, FD], FR, tag="F4b", bufs=2, name="F4b")
        mlp_layer([F3a[:], F3b[:]], [cs["w4c0"], cs["w4c1"]],
                  [cs["b4c0"], cs["b4c1"]], [F4a, F4b], "L4")
        return F4a, F4b

    # quarter-granularity software pipeline: round `it` emits
    # head(it+2), L1(it+1), L3(it), L2(it+1), L4(it), tail(it-1)
    head_of = {0: heads, 1: nexts}
    F1_of = {0: emit_L1(head_of[0])}
    Y_of = {0: emit_L2(*F1_of.pop(0))}
    F4_of = {}
    for it in range(N_ITERS):
        if it + 2 < N_ITERS:
            head_of[it + 2] = stage_head(it + 2)
        if it + 1 < N_ITERS:
            F1_of[it + 1] = emit_L1(head_of[it + 1])
        if it >= 1:
            body_tail(it - 1, *F4_of.pop(it - 1))
        F3s = emit_L3(head_of[it], *Y_of.pop(it))
        if it + 1 < N_ITERS:
            Y_of[it + 1] = emit_L2(*F1_of.pop(it + 1))
        F4_of[it] = emit_L4(*F3s)
        head_of.pop(it)
    body_tail(N_ITERS - 1, *F4_of.pop(N_ITERS - 1))

    for p in reversed(main_ctx):
        p.__exit__(None, None, None)

    tc.no_sync_barrier()

    # ================= stage 3: alphas (exp/ln table) ======================
    ep = pool("ep", 1)
    epps = pool("epps", 2, "PSUM")

    z_all = ep.tile([128, 512], FP)
    nc.sync.dma_start(z_all[:], h["z_scr"][:].rearrange("(p f) -> p f", f=512))
    t1 = ep.tile([128, 512], FP)
    nc.scalar.activation(t1[:], z_all[:], A.Abs, bias=cs["bam1"][:, 0:1])
    e1 = ep.tile([128, 512], FP)
    nc.scalar.activation(e1[:], t1[:], A.Exp, scale=-1.0)
    l1 = ep.tile([128, 512], FP)
    nc.scalar.activation(l1[:], e1[:], A.Ln, bias=1.0)
    r1 = ep.tile([128, 512], FP)
    nc.scalar.activation(r1[:], z_all[:], A.Relu, bias=cs["bam1"][:, 0:1])
    alphas = ep.tile([128, 512], FP)
    nc.vector.tensor_add(alphas[:], l1[:], r1[:])
    aw = ep.tile([128, 512], FP)
    nc.vector.tensor_mul(aw[:], alphas[:], weights_all[:].bitcast(FP))
    apred = ep.tile([128, 32], FP)
    nc.vector.tensor_reduce(apred[:], aw[:].rearrange("p (a b) -> p a b", b=16),
                            axis=mybir.AxisListType.X, op=OP.add)
    nc.sync.dma_start(
        out_h[0:1, :].rearrange("o (p f) -> (o p) f", f=32), apred[:])

    # ================= stage 4: color MLP (exp table) ======================
    CFD = 1024
    for c in range(N_PER // CFD):
        cslc = bass.ds(c * CFD, CFD)
        erc2 = ep.tile([24, CFD], FR, tag="erc2", bufs=2, name="erc2")
        nc.sync.dma_start(erc2[:], h["er_scr"][:, cslc].bitcast(FR))
        psC1 = epps.tile([128, CFD], FP, tag="psC")
        for n in range(2):
            nslc = bass.ds(c * CFD + n * 512, 512)
            pslc = bass.ts(n, 512)
            for ci, (src2, wt) in enumerate((
                    (FA0[:, nslc], cs["wc1c0"]),
                    (FA1[:, nslc], cs["wc1c1"]),
                    (erc2[:, pslc], cs["wc1c2"]))):
                nc.tensor.matmul(psC1[:, pslc], wt[:], src2,
                                 start=(ci == 0), stop=(ci == 2))
        H1 = ep.tile([128, CFD], FR, tag="H1")
        nc.scalar.activation(H1[:], psC1[:], A.Prelu,
                             bias=cs["bc1"][:, 0:1], alpha=0.01)
        psC2 = epps.tile([128, CFD], FP, tag="psC")
        for n in range(2):
            pslc = bass.ts(n, 512)
            nc.tensor.matmul(psC2[:, pslc], cs["wc2"][:], H1[:, pslc],
                             start=True, stop=True)
        H2 = ep.tile([128, CFD], FR, tag="H2")
        nc.scalar.activation(H2[:], psC2[:], A.Prelu,
                             bias=cs["bc2"][:, 0:1], alpha=0.01)
        psC3 = epps.tile([128, CFD], FP, tag="psC")
        for n in range(2):
            pslc = bass.ts(n, 512)
            nc.tensor.matmul(psC3[:, pslc], cs["wc3"][:], H2[:, pslc],
                             start=True, stop=True)
        H3 = ep.tile([128, CFD], FR, tag="H3")
        nc.scalar.activation(H3[:], psC3[:], A.Prelu,
                             bias=cs["bc3"][:, 0:1], alpha=0.01)
        psC4 = epps.tile([3, CFD], FP, tag="psC4")
        for n in range(2):
            pslc = bass.ts(n, 512)
            nc.tensor.matmul(psC4[:, pslc], cs["wc4"][:].bitcast(FP),
                             H3[:, pslc].bitcast(FP), start=True, stop=True)
        ec = ep.tile([3, CFD], FP, tag="ec")
        nc.scalar.activation(ec[:], psC4[:], A.Exp, scale=-1.0,
                             bias=cs["negbc4"][:, 0:1])
        nc.vector.tensor_scalar_add(ec[:], ec[:], 1.0)
        rc = ep.tile([3, CFD], FP, tag="rc")
        nc.vector.reciprocal(rc[:], ec[:])
        nc.vector.tensor_scalar(rc[:], rc[:], 1.002, -0.001,
                                op0=OP.mult, op1=OP.add)
        nc.sync.dma_start(out_h[1:4, cslc], rc[:])

    for p in reversed(ctxs):
        p.__exit__(None, None, None)


# ---------------------------------------------------------------- entry point
def _get_nc():
    if "nc" not in _CACHE:
        _CACHE["nc"] = _build()
    return _CACHE["nc"]


def _run(inputs, trace=False):
    nc = _get_nc()
    consts = _consts(inputs)
    shards = _shards(inputs)
    in_maps = [{**consts, **sh} for sh in shards]
    res = run_bass_kernel_spmd(nc, in_maps, core_ids=list(range(N_CORES)),
                               trace=trace)
    outs = [r["out"] for r in res.results]
    alpha = np.concatenate([o[0] for o in outs]).astype(np.float32)
    color = np.concatenate([o[1:4].T for o in outs]).astype(np.float32)
    B, R = 32, 1024
    return (alpha.reshape(B, R), color.reshape(B, R, 3)), res


def kernel(**inputs):
    out, _ = _run(inputs, trace=False)
    return out
